# revision 5
# baseline (speedup 1.0000x reference)
"""Trainium2 Bass kernel for nn_CvtNodeInitializer (gnn_message_passing).

Strategy (v2):
  - Host: keep only edges whose tail is a CVT node. Bin-pack CVT nodes
    into windows (caps: 128 nodes AND 128 edge slots per window) so one
    window = one 128-slot matmul tile. Deal windows contiguously to the
    8 cores. Route each edge's gathered feature row
    [relation_tokens[e] | node_tokens[e]] (the reference's edge-slot
    quirk) to its window, pre-transposed into matmul lhsT layout, fp16.
  - Device (SPMD): per window
      pm[slot, 0:257] = X_win @ [W_msg.T | W_msg.T @ attn]   (4 fp16 matmuls)
      q = exp(pm[:, 256])                                    (Act engine)
      ms = [fp16(pm[:, 0:256]) | ones]                       (Act engine)
      oh[slot, node] = (iota == seg) * q                     (DVE)
      pg[node, 0:257] = oh.T @ ms  -> [agg | den]            (1 fp16 matmul)
      out tile = bf16(pg)                                    (Pool engine)
    Windows are processed in pairs sharing 2-bank PSUM tiles so the
    small ops batch across both windows of a pair (strided APs).
  - Host: comp = agg / max(den, tiny) + shared_cvt, scatter into the
    CVT rows of a copy of node_tokens. Non-CVT rows are exact f32.
"""

import sys

sys.path.insert(0, "/opt/trn_rl_repo")

import numpy as np

N_NODES = 200000
N_EDGES = 200000
HID = 256
NCORES = 8
P = 128
HC = HID + 1  # 256 msg cols + 1 logit/den col

_PROGRAM_CACHE: dict = {}


def _build_program(S: int, W: int, repeats: int = 1):
    """Per-core Bass program. S is fixed at 128 (one tile per window),
    W = windows per core (even; processed in pairs)."""
    import concourse.bacc as bacc
    import concourse.mybir as mybir
    import concourse.tile as tile

    f32 = mybir.dt.float32
    f16 = mybir.dt.float16
    bf16 = mybir.dt.bfloat16
    i32 = mybir.dt.int32
    Alu = mybir.AluOpType
    Act = mybir.ActivationFunctionType

    assert S == P and W % 2 == 0
    W2 = W // 2

    nc = bacc.Bacc()
    xt = nc.declare_dram_parameter("xt", [W2, P, 2, 4 * P], f16, isOutput=False)
    sc = nc.declare_dram_parameter("sc", [P, W], f32, isOutput=False)
    wq = nc.declare_dram_parameter("wq", [P, 4, HC], f16, isOutput=False)
    out = nc.declare_dram_parameter("out", [W2, P, 2, HC], bf16, isOutput=True)

    with tile.TileContext(nc) as tc:
        with (
            tc.tile_pool(name="const", bufs=1) as cpool,
            tc.tile_pool(name="x", bufs=4) as xpool,
            tc.tile_pool(name="oh", bufs=4) as opool,
            tc.tile_pool(name="od", bufs=4) as dpool,
            tc.tile_pool(name="q", bufs=4) as qpool,
            tc.tile_pool(name="pm", bufs=2, space="PSUM") as pmpool,
            tc.tile_pool(name="pg", bufs=2, space="PSUM") as pgpool,
        ):
            # --- one-time constants ---
            wtile = cpool.tile([P, 4, HC], f16)
            sctile = cpool.tile([P, W], f32)
            io_i = cpool.tile([P, P], i32)
            io_f = cpool.tile([P, P], f32)
            nc.sync.dma_start(out=wtile[:], in_=wq[:])
            nc.sync.dma_start(out=sctile[:], in_=sc[:])
            nc.gpsimd.iota(io_i[:], pattern=[[1, P]], base=0, channel_multiplier=0)
            nc.vector.tensor_copy(io_f[:], io_i[:])
            # ms ring: [slot, pair-window, 257]; col 256 preset to 1.0 (den)
            NMS = 3
            msring = [
                cpool.tile([P, 2, HC], f16, name=f"ms{i}") for i in range(NMS)
            ]
            for m in msring:
                nc.gpsimd.memset(m[:, :, HID], 1.0)

            def front(g, ms):
                """DMA in, msg matmuls, exp, ms copy, one-hot build."""
                xg = xpool.tile([P, 2, 4 * P], f16, tag="xg")
                nc.sync.dma_start(out=xg[:], in_=xt[g])
                pm = pmpool.tile([P, 2, 2 * HID], f32, tag="pm")
                for k in range(2):
                    for c in range(4):
                        nc.tensor.matmul(
                            pm[:, k, 0:HC],
                            lhsT=xg[:, k, c * P:(c + 1) * P],
                            rhs=wtile[:, c, :],
                            start=(c == 0),
                            stop=(c == 3),
                        )
                # q = exp(logit) for both windows, one Act instr
                q = qpool.tile([P, 2], f32, tag="q")
                nc.scalar.activation(q[:], pm[:, :, HID], Act.Exp)
                # ms[:, k, 0:256] = fp16(pm[:, k, 0:256]); col 256 stays 1.0
                nc.scalar.activation(ms[:, :, 0:HID], pm[:, :, 0:HID], Act.Copy)
                ohs = []
                for k in range(2):
                    oh = opool.tile([P, P], f16, tag="oh")
                    nc.vector.tensor_scalar(
                        out=oh[:],
                        in0=io_f[:],
                        scalar1=sctile[:, 2 * g + k:2 * g + k + 1],
                        scalar2=q[:, k:k + 1],
                        op0=Alu.is_equal,
                        op1=Alu.mult,
                    )
                    ohs.append(oh)
                return (g, ms, ohs)

            def back(st):
                """Agg matmuls, bf16 narrow, DMA out (Act hwdge queue)."""
                g, ms, ohs = st
                pg = pgpool.tile([P, 2, 2 * HID], f32, tag="pg")
                for k in range(2):
                    nc.tensor.matmul(
                        pg[:, k, 0:HC],
                        lhsT=ohs[k][:],
                        rhs=ms[:, k, :],
                        start=True,
                        stop=True,
                    )
                od = dpool.tile([P, 2, HC], bf16, tag="od")
                nc.vector.tensor_copy(od[:], pg[:, :, 0:HC])
                nc.scalar.dma_start(out=out[g], in_=od[:])

            def all_pairs():
                # software pipeline: agg of pair g-1 is emitted after the
                # msg matmuls of pair g, so the PE never waits on the
                # exp -> one-hot chain
                pend = None
                for g in range(W2):
                    st = front(g, msring[g % NMS])
                    if pend is not None:
                        back(pend)
                    pend = st
                back(pend)

            if repeats == 1:
                all_pairs()
            else:
                with tc.For_i(0, repeats, 1) as _iv:
                    all_pairs()

    nc.compile()
    return nc


def _pack(tails, cvt):
    """Greedy-pack CVT nodes into windows (<=128 nodes, <=128 edges).
    Returns (node_win, node_slot, cvt_nodes, TW)."""
    cvt_nodes = np.nonzero(cvt)[0]
    ncvt = len(cvt_nodes)
    deg_all = np.bincount(tails[cvt[tails]], minlength=len(cvt))
    deg = deg_all[cvt_nodes]
    assert deg.max() <= P, "single node exceeds window edge capacity"
    cum = np.zeros(ncvt + 1, dtype=np.int64)
    np.cumsum(deg, out=cum[1:])
    bounds = []
    s = 0
    while s < ncvt:
        e_end = int(np.searchsorted(cum, cum[s] + P, side="right")) - 1
        end = min(s + P, e_end, ncvt)
        assert end > s
        bounds.append(end)
        s = end
    TW = len(bounds)
    starts = np.empty(TW, dtype=np.int64)
    starts[0] = 0
    starts[1:] = bounds[:-1]
    lens = np.asarray(bounds, dtype=np.int64) - starts
    node_win = np.repeat(np.arange(TW, dtype=np.int64), lens)
    node_slot = np.arange(ncvt, dtype=np.int64) - starts[node_win]
    return node_win, node_slot, cvt_nodes, TW


def _host_prep_full(node_tokens, relation_tokens, edge_index, node_is_cvt,
                    shared_cvt, attn_vector, W_msg, n_cores=NCORES):
    node_tokens = np.asarray(node_tokens, np.float32)
    relation_tokens = np.asarray(relation_tokens, np.float32)
    tails = np.asarray(edge_index[1], dtype=np.int64)
    cvt = np.asarray(node_is_cvt, dtype=bool)
    hid = node_tokens.shape[1]

    node_win, node_slot, cvt_nodes, TW = _pack(tails, cvt)

    # pad total windows to a multiple of 2*n_cores (pairs, SPMD)
    Wp = ((TW + 2 * n_cores - 1) // (2 * n_cores)) * 2
    TWp = Wp * n_cores

    # edge -> (window, seg, rank)
    inv = np.zeros(len(cvt), dtype=np.int64)
    inv[cvt_nodes] = np.arange(len(cvt_nodes))
    eids = np.nonzero(cvt[tails])[0]
    et = inv[tails[eids]]
    ewin = node_win[et]
    eseg = node_slot[et]
    order = np.argsort(ewin, kind="stable")
    eids, ewin, eseg = eids[order], ewin[order], eseg[order]
    cnt = np.bincount(ewin, minlength=TWp)
    wstart = np.zeros(TWp, dtype=np.int64)
    np.cumsum(cnt[:-1], out=wstart[1:])
    rank = np.arange(len(ewin)) - wstart[ewin]

    # gathered edge features, fp16, scattered into [TWp, 128 slots, 512]
    xall = np.zeros((TWp, P, 2 * hid), np.float16)
    xall[ewin, rank, :hid] = relation_tokens[eids]
    xall[ewin, rank, hid:] = node_tokens[eids]
    # -> lhsT chunk layout [TWp, 128 part, 4*128]: [p, c*128+j] = X[j, c*128+p]
    xall = (xall.transpose(0, 2, 1)
            .reshape(TWp, 4, P, P)
            .transpose(0, 2, 1, 3)
            .reshape(n_cores, Wp // 2, 2, P, 4 * P)
            .transpose(0, 1, 3, 2, 4))
    xt_all = np.ascontiguousarray(xall)

    segf = np.full((TWp, P), -1000.0, np.float32)
    segf[ewin, rank] = eseg.astype(np.float32)
    sc_all = np.ascontiguousarray(
        segf.reshape(n_cores, Wp, P).transpose(0, 2, 1)
    )

    Wt = np.asarray(W_msg, np.float32).T  # [2H, H]
    va = Wt @ np.asarray(attn_vector, np.float32)  # [2H]
    waug = np.concatenate([Wt, va[:, None]], axis=1)  # [2H, 257]
    wq = np.ascontiguousarray(
        waug.reshape(4, P, HC).transpose(1, 0, 2)
    ).astype(np.float16)

    in_maps = [
        {"xt": xt_all[c], "sc": sc_all[c], "wq": wq}
        for c in range(n_cores)
    ]
    meta = (node_win, node_slot, cvt_nodes, Wp)
    return in_maps, meta


def _host_prep(node_tokens, relation_tokens, edge_index, node_is_cvt,
               shared_cvt, attn_vector, W_msg, n_cores=NCORES):
    """test.py timing contract: returns (in_maps, S, W, npc)."""
    in_maps, meta = _host_prep_full(
        node_tokens, relation_tokens, edge_index, node_is_cvt,
        shared_cvt, attn_vector, W_msg, n_cores)
    return in_maps, P, meta[3], N_NODES // n_cores


def kernel(**inputs) -> np.ndarray:
    from concourse import bass2jax

    node_tokens = np.asarray(inputs["node_tokens"], np.float32)
    in_maps, meta = _host_prep_full(
        node_tokens,
        inputs["relation_tokens"],
        inputs["edge_index"],
        inputs["node_is_cvt"],
        inputs["shared_cvt"],
        inputs["attn_vector"],
        inputs["W_msg"],
    )
    node_win, node_slot, cvt_nodes, Wp = meta
    key = (P, Wp)
    nc = _PROGRAM_CACHE.get(key)
    if nc is None:
        nc = _build_program(P, Wp)
        _PROGRAM_CACHE[key] = nc
    results = bass2jax.run_bass_via_pjrt(nc, in_maps, n_cores=len(in_maps))

    hid = node_tokens.shape[1]
    # [W2, 128, 2, 257] per core -> global [TWp, 128, 257] f32
    allw = np.concatenate(
        [r["out"].transpose(0, 2, 1, 3).reshape(Wp, P, HC) for r in results],
        axis=0,
    ).astype(np.float32)
    agg = allw[:, :, :hid]
    den = allw[:, :, hid]
    comp = agg / np.where(den > 0, den, 1.0)[:, :, None]
    comp += np.asarray(inputs["shared_cvt"], np.float32)[None, None, :]

    out_full = node_tokens.copy()
    out_full[cvt_nodes] = comp[node_win, node_slot]
    return out_full


# revision 17
# speedup vs baseline: 6.4912x; 6.4912x over previous
"""Trainium2 Bass kernel for nn_CvtNodeInitializer (gnn_message_passing).

Strategy (v4):
  - Host: keep only edges whose tail is a CVT node. Bin-pack CVT nodes
    into windows (caps: 128 nodes AND 128 edge slots per window) so one
    window = one 128-slot matmul tile. Deal windows contiguously to the
    8 cores. Route each edge's gathered feature row
    [relation_tokens[e] | node_tokens[e]] (the reference's edge-slot
    quirk) to its window, pre-transposed into matmul lhsT layout, fp16.
  - Device (SPMD): windows processed in pairs sharing 2-bank PSUM
    tiles (batched exp / msg-narrow); DMA in groups of K_G windows
    (large descriptors; in on SP hwdge queue, out on Act hwdge queue):
      pm[slot, 0:257] = X_win @ [W_msg.T | W_msg.T @ attn]   (4 fp16 matmuls)
      q = exp(pm[:, 256])                                    (Act)
      ms = [fp16(pm[:, 0:256]) | ones]                       (Act)
      oh[slot, node] = (iota == seg) * q                     (DVE)
      pg[node, 0:257] = oh.T @ ms  -> [agg | den]            (1 fp16 matmul)
      od group tile = bf16(pg)                               (DVE)
    The agg matmul of pair i is emitted after the msg matmuls of pair
    i+1 so the PE never stalls on the exp -> one-hot chain.
  - Host: comp = agg / max(den, tiny) + shared_cvt, scatter into the
    CVT rows of a copy of node_tokens. Non-CVT rows are exact f32.
"""

import os
import sys

sys.path.insert(0, "/opt/trn_rl_repo")

import numpy as np

N_NODES = 200000
N_EDGES = 200000
HID = 256
NCORES = 8
P = 128
HC = HID + 1  # 256 msg cols + 1 logit/den col

K_G = int(os.environ.get("K_G", "8"))  # windows per DMA group (even)
K_ABL = os.environ.get("K_ABL", "")    # "", "dma", "dmain", "dmaout"

_PROGRAM_CACHE: dict = {}


def _build_program(S: int, W: int, repeats: int = 1):
    """Per-core Bass program. S fixed at 128 (one tile per window),
    W = windows per core (multiple of K_G)."""
    import concourse.bacc as bacc
    import concourse.mybir as mybir
    import concourse.tile as tile

    f32 = mybir.dt.float32
    f16 = mybir.dt.float16
    bf16 = mybir.dt.bfloat16
    i32 = mybir.dt.int32
    Alu = mybir.AluOpType
    Act = mybir.ActivationFunctionType

    G = K_G
    assert S == P and W % G == 0 and G % 2 == 0
    WG = W // G
    NP2 = G // 2  # pairs per group

    nc = bacc.Bacc()
    xt = nc.declare_dram_parameter("xt", [WG, P, G * 4 * P], f16,
                                   isOutput=False)
    sc = nc.declare_dram_parameter("sc", [P, W], f32, isOutput=False)
    wq = nc.declare_dram_parameter("wq", [P, 4, HC], f16, isOutput=False)
    out = nc.declare_dram_parameter("out", [WG, P, G * HC], bf16,
                                    isOutput=True)

    with tile.TileContext(nc) as tc:
        with (
            tc.tile_pool(name="const", bufs=1) as cpool,
            tc.tile_pool(name="x", bufs=2) as xpool,
            tc.tile_pool(name="oh", bufs=4) as opool,
            tc.tile_pool(name="od", bufs=2) as dpool,
            tc.tile_pool(name="q", bufs=4) as qpool,
            tc.tile_pool(name="pm", bufs=2, space="PSUM") as pmpool,
            tc.tile_pool(name="pg", bufs=2, space="PSUM") as pgpool,
        ):
            # --- one-time constants ---
            wtile = cpool.tile([P, 4, HC], f16)
            sctile = cpool.tile([P, W], f32)
            io_i = cpool.tile([P, P], i32)
            io_f = cpool.tile([P, P], f32)
            nc.sync.dma_start(out=wtile[:], in_=wq[:])
            nc.sync.dma_start(out=sctile[:], in_=sc[:])
            nc.gpsimd.iota(io_i[:], pattern=[[1, P]], base=0,
                           channel_multiplier=0)
            nc.vector.tensor_copy(io_f[:], io_i[:])
            # ms ring: [slot, pair-window, 257]; col 256 preset to 1.0 (den)
            NMS = 3
            msring = [
                cpool.tile([P, 2, HC], f16, name=f"ms{i}") for i in range(NMS)
            ]
            for m in msring:
                nc.gpsimd.memset(m[:, :, HID], 1.0)

            def group_in(gr):
                xg = xpool.tile([P, G, 4 * P], f16, tag="xg")
                nc.sync.dma_start(out=xg[:], in_=xt[gr])
                od8 = dpool.tile([P, G, HC], bf16, tag="od8")
                return xg, od8

            def front_pe(xg, pj):
                """msg matmuls for pair pj of the current group."""
                pm = pmpool.tile([P, 2, 2 * HID], f32, tag="pm")
                for k in range(2):
                    for c in range(4):
                        nc.tensor.matmul(
                            pm[:, k, 0:HC],
                            lhsT=xg[:, 2 * pj + k, c * P:(c + 1) * P],
                            rhs=wtile[:, c, :],
                            start=(c == 0),
                            stop=(c == 3),
                        )
                return pm

            def front_post(w0, ms, pm):
                """exp, ms copy, one-hot build (w0 = first window id)."""
                q = qpool.tile([P, 2], f32, tag="q")
                nc.scalar.activation(q[:], pm[:, :, HID], Act.Exp)
                nc.scalar.activation(ms[:, :, 0:HID], pm[:, :, 0:HID],
                                     Act.Copy)
                ohs = []
                for k in range(2):
                    oh = opool.tile([P, P], f16, tag="oh")
                    nc.vector.tensor_scalar(
                        out=oh[:],
                        in0=io_f[:],
                        scalar1=sctile[:, w0 + k:w0 + k + 1],
                        scalar2=q[:, k:k + 1],
                        op0=Alu.is_equal,
                        op1=Alu.mult,
                    )
                    ohs.append(oh)
                return ohs

            def back(st):
                """Agg matmuls, bf16 narrow into the group tile; issue the
                group's out-DMA (Act hwdge queue) after its last pair."""
                gr, pj, ms, ohs, od8 = st
                pg = pgpool.tile([P, 2, 2 * HID], f32, tag="pg")
                for k in range(2):
                    nc.tensor.matmul(
                        pg[:, k, 0:HC],
                        lhsT=ohs[k][:],
                        rhs=ms[:, k, :],
                        start=True,
                        stop=True,
                    )
                nc.vector.tensor_copy(
                    od8[:, 2 * pj:2 * pj + 2, :], pg[:, :, 0:HC]
                )
                if pj == NP2 - 1:
                    nc.scalar.dma_start(out=out[gr], in_=od8[:])

            def abl_pairs():
                """Ablation variants (timing only)."""
                for gr in range(WG):
                    if K_ABL in ("dma", "dmain"):
                        xg = xpool.tile([P, G, 4 * P], f16, tag="xg")
                        nc.sync.dma_start(out=xg[:], in_=xt[gr])
                    if K_ABL in ("dma", "dmaout"):
                        od8 = dpool.tile([P, G, HC], bf16, tag="od8")
                        nc.gpsimd.memset(od8[:, 0, 0:1], 0.0)
                        nc.scalar.dma_start(out=out[gr], in_=od8[:])

            def all_pairs():
                if K_ABL:
                    abl_pairs()
                    return
                pend = None
                g2 = 0
                for gr in range(WG):
                    xg, od8 = group_in(gr)
                    for pj in range(NP2):
                        pm = front_pe(xg, pj)
                        if pend is not None:
                            back(pend)
                        ms = msring[g2 % NMS]
                        ohs = front_post(gr * G + 2 * pj, ms, pm)
                        pend = (gr, pj, ms, ohs, od8)
                        g2 += 1
                back(pend)

            if repeats == 1:
                all_pairs()
            else:
                with tc.For_i(0, repeats, 1) as _iv:
                    all_pairs()

    nc.compile()
    return nc


def _pack(tails, cvt):
    """Greedy-pack CVT nodes into windows (<=128 nodes, <=128 edges).
    Returns (node_win, node_slot, cvt_nodes, TW)."""
    cvt_nodes = np.nonzero(cvt)[0]
    ncvt = len(cvt_nodes)
    deg_all = np.bincount(tails[cvt[tails]], minlength=len(cvt))
    deg = deg_all[cvt_nodes]
    assert deg.max() <= P, "single node exceeds window edge capacity"
    cum = np.zeros(ncvt + 1, dtype=np.int64)
    np.cumsum(deg, out=cum[1:])
    bounds = []
    s = 0
    while s < ncvt:
        e_end = int(np.searchsorted(cum, cum[s] + P, side="right")) - 1
        end = min(s + P, e_end, ncvt)
        assert end > s
        bounds.append(end)
        s = end
    TW = len(bounds)
    starts = np.empty(TW, dtype=np.int64)
    starts[0] = 0
    starts[1:] = bounds[:-1]
    lens = np.asarray(bounds, dtype=np.int64) - starts
    node_win = np.repeat(np.arange(TW, dtype=np.int64), lens)
    node_slot = np.arange(ncvt, dtype=np.int64) - starts[node_win]
    return node_win, node_slot, cvt_nodes, TW


def _host_prep_full(node_tokens, relation_tokens, edge_index, node_is_cvt,
                    shared_cvt, attn_vector, W_msg, n_cores=NCORES):
    node_tokens = np.asarray(node_tokens, np.float32)
    relation_tokens = np.asarray(relation_tokens, np.float32)
    tails = np.asarray(edge_index[1], dtype=np.int64)
    cvt = np.asarray(node_is_cvt, dtype=bool)
    hid = node_tokens.shape[1]

    node_win, node_slot, cvt_nodes, TW = _pack(tails, cvt)

    # pad total windows so each core gets a multiple of K_G (SPMD)
    Wp = ((TW + K_G * n_cores - 1) // (K_G * n_cores)) * K_G
    TWp = Wp * n_cores
    WG = Wp // K_G

    # edge -> (window, seg, rank)
    inv = np.zeros(len(cvt), dtype=np.int64)
    inv[cvt_nodes] = np.arange(len(cvt_nodes))
    eids = np.nonzero(cvt[tails])[0]
    et = inv[tails[eids]]
    ewin = node_win[et]
    eseg = node_slot[et]
    order = np.argsort(ewin, kind="stable")
    eids, ewin, eseg = eids[order], ewin[order], eseg[order]
    cnt = np.bincount(ewin, minlength=TWp)
    wstart = np.zeros(TWp, dtype=np.int64)
    np.cumsum(cnt[:-1], out=wstart[1:])
    rank = np.arange(len(ewin)) - wstart[ewin]

    # gathered edge features, fp16, scattered into [TWp, 128 slots, 512]
    xall = np.zeros((TWp, P, 2 * hid), np.float16)
    xall[ewin, rank, :hid] = relation_tokens[eids]
    xall[ewin, rank, hid:] = node_tokens[eids]
    # -> lhsT chunk layout [p, c*128+j] = X[j, c*128+p], grouped by K_G
    xall = (xall.transpose(0, 2, 1)
            .reshape(TWp, 4, P, P)
            .transpose(0, 2, 1, 3)
            .reshape(n_cores, WG, K_G, P, 4 * P)
            .transpose(0, 1, 3, 2, 4)
            .reshape(n_cores, WG, P, K_G * 4 * P))
    xt_all = np.ascontiguousarray(xall)

    segf = np.full((TWp, P), -1000.0, np.float32)
    segf[ewin, rank] = eseg.astype(np.float32)
    sc_all = np.ascontiguousarray(
        segf.reshape(n_cores, Wp, P).transpose(0, 2, 1)
    )

    Wt = np.asarray(W_msg, np.float32).T  # [2H, H]
    va = Wt @ np.asarray(attn_vector, np.float32)  # [2H]
    waug = np.concatenate([Wt, va[:, None]], axis=1)  # [2H, 257]
    wq = np.ascontiguousarray(
        waug.reshape(4, P, HC).transpose(1, 0, 2)
    ).astype(np.float16)

    in_maps = [
        {"xt": xt_all[c], "sc": sc_all[c], "wq": wq}
        for c in range(n_cores)
    ]
    meta = (node_win, node_slot, cvt_nodes, Wp)
    return in_maps, meta


def _host_prep(node_tokens, relation_tokens, edge_index, node_is_cvt,
               shared_cvt, attn_vector, W_msg, n_cores=NCORES):
    """test.py timing contract: returns (in_maps, S, W, npc)."""
    in_maps, meta = _host_prep_full(
        node_tokens, relation_tokens, edge_index, node_is_cvt,
        shared_cvt, attn_vector, W_msg, n_cores)
    return in_maps, P, meta[3], N_NODES // n_cores


def kernel(**inputs) -> np.ndarray:
    from concourse import bass2jax

    node_tokens = np.asarray(inputs["node_tokens"], np.float32)
    in_maps, meta = _host_prep_full(
        node_tokens,
        inputs["relation_tokens"],
        inputs["edge_index"],
        inputs["node_is_cvt"],
        inputs["shared_cvt"],
        inputs["attn_vector"],
        inputs["W_msg"],
    )
    node_win, node_slot, cvt_nodes, Wp = meta
    key = (P, Wp, K_G, K_ABL)
    nc = _PROGRAM_CACHE.get(key)
    if nc is None:
        nc = _build_program(P, Wp)
        _PROGRAM_CACHE[key] = nc
    results = bass2jax.run_bass_via_pjrt(nc, in_maps, n_cores=len(in_maps))

    hid = node_tokens.shape[1]
    # [WG, P, G*257] per core -> global [TWp, 128, 257] f32
    WG = Wp // K_G
    allw = np.concatenate(
        [r["out"].reshape(WG, P, K_G, HC).transpose(0, 2, 1, 3)
         .reshape(Wp, P, HC) for r in results],
        axis=0,
    ).astype(np.float32)
    agg = allw[:, :, :hid]
    den = allw[:, :, hid]
    comp = agg / np.where(den > 0, den, 1.0)[:, :, None]
    comp += np.asarray(inputs["shared_cvt"], np.float32)[None, None, :]

    out_full = node_tokens.copy()
    out_full[cvt_nodes] = comp[node_win, node_slot]
    return out_full
